# revision 7
# baseline (speedup 1.0000x reference)
"""Trainium2 Bass kernel for nn_CAM_85770496901546 (sparse_attention).

Data-parallel over batch: 16 batch elements -> 8 cores x 2.

Per batch element (P=32 patch grid, 8x8 patches, c=64 channels):
  pfb   = maxpool8x8(mask)                      [1024]
  f     = avgpool2x2(feature_attn) flattened    [128 c, 1024 patches] (x0.25
          scale omitted: cancels in cosine normalization)
  cmat  = cos(i,j) * pfb[i] * (1-pfb[j])
  s     = softmax_j(cmat) * p_matrix
  out   = s @ fp,  fp = patch-gathered feature  [1024 j, 4096 d]

Everything on device is computed in the transposed [j, i] layout so softmax
denominators / per-i factors fold into matmuls and PSUM evacuation (no
on-chip transposes at all):
  fT2[c,i]  = fT_bf[c,i] * b[i], b = rnorm*pfb  (b broadcast via K=1 matmul;
              folding it into f makes sim2 = f^T fT2 = sim * b[i] directly)
  E[j,i]    = exp(sim2 * a[j]),  a = rnorm*(1-pfb) as per-partition ACT scale
              (ACT reads the sim2 PSUM tile directly, writes bf16)
  D[i]      = sum_j E  (ones-column matmul, fp32 PSUM accumulation)
  sT_eff    = E * (1-pfb[j])            (per-partition tensor_scalar, bf16)
  out[i,d]  = (sum_j sT_eff[j,i] fp[j,d]) * (pfb[i]/D[i])  <- folded into the
              PSUM->SBUF evacuation tensor_scalar

Phase ordering keeps TensorE dense for HAM warmth: prep+softmax for BOTH
batch elements runs before/overlapping the two back-to-back main-matmul
blocks (batch 1's softmax overlaps batch 0's main matmul; PSUM is split
2 banks for the matmul accumulators + 6 banks for the softmax pipeline).

The patch gather of `feature` -> fp[j, d] and the inverse scatter of the
output are pure data-movement permutations of the sharding layer; they are
done on host in numpy (fp is also pre-cast to bf16 there, halving its HBM
footprint). Exp needs no max-subtraction: |cmat| <= 1 by construction.
"""

import numpy as np
import ml_dtypes

import concourse.bacc as bacc
import concourse.tile as tile
import concourse.mybir as mybir
from concourse.bass_utils import run_bass_kernel_spmd

F32 = mybir.dt.float32
BF16 = mybir.dt.bfloat16
AX = mybir.AxisListType
OP = mybir.AluOpType
ACT = mybir.ActivationFunctionType

N_CORES = 8
BPC = 2          # batch elements per core
P = 32           # patch grid
NP = P * P       # 1024 patches
C = 64           # feature channels
D = 4096         # ph*pw*c
CA = 128         # attn channels


def _emit_loads(nc, b, io, pools, state):
    fp_in, fa_in, mask_in, out_dev = io
    fpp, ldp, stp, per, wk, cst = pools
    mask_t = ldp.tile([32, 2048], F32, tag="mask", bufs=1)
    nc.sync.dma_start(mask_t[:], mask_in[b].rearrange("(a q) w -> a (q w)", q=8))
    fa_t = ldp.tile([CA, 4096], F32, tag="fa", bufs=1)
    nc.sync.dma_start(fa_t[:], fa_in[b])
    fpt = []
    for jb in range(8):
        for q in range(4):
            t = fpp.tile([128, 1024], BF16, tag="fp")
            nc.sync.dma_start(
                t[:], fp_in[b, jb * 128:(jb + 1) * 128,
                             q * 1024:(q + 1) * 1024])
            fpt.append(t)  # index jb*4 + q
    state[b] = {"mask_t": mask_t, "fa_t": fa_t, "fpt": fpt}


def _emit_softmax(nc, tc, b, pools, state, consts):
    """Phase 0+1: pfb, f, sim, exp, D, sT, g."""
    fpp, ldp, stp, per, wk, cst = pools
    ones_col_f, ones_col_b, ones_row = consts
    st_ = state[b]
    mask_t, fa_t = st_["mask_t"], st_["fa_t"]

    with tc.tile_pool(name=f"pp0_{b}", bufs=1, space="PSUM") as pp0, \
         tc.tile_pool(name=f"p1s_{b}", bufs=(2 if b == 0 else 1),
                      space="PSUM") as sp, \
         tc.tile_pool(name=f"p1d_{b}", bufs=1, space="PSUM") as dp:
        # row vectors (separate tiles: matmul operands need base partition 0)
        pfb_row = per.tile([1, NP], F32, tag="pfbr", bufs=1)
        rnorm_row = per.tile([1, NP], F32, tag="rnr", bufs=1)
        b_row = per.tile([1, NP], F32, tag="brow", bufs=1)
        g_row = per.tile([1, NP], F32, tag="grow", bufs=1)
        srt = wk.tile([1, NP], F32, tag="srt", bufs=1)
        dsb = wk.tile([1, NP], F32, tag="dsb", bufs=1)
        rdr = wk.tile([1, NP], F32, tag="rdr", bufs=1)

        # mask maxpool -> pfb row
        m1 = wk.tile([32, 256], F32, tag="m1", bufs=1)
        nc.vector.tensor_reduce(
            m1[:], mask_t.rearrange("p (ph w) -> p w ph", ph=8), AX.X, OP.max)
        pfb2d = wk.tile([32, 32], F32, tag="m2", bufs=1)
        nc.vector.tensor_reduce(
            pfb2d[:], m1.rearrange("p (pw q) -> p pw q", q=8), AX.X, OP.max)
        nc.sync.dma_start(pfb_row[:], pfb2d[:])

        # feature_attn avgpool (no 0.25 scale) + bf16 cast + squares
        fav = fa_t.rearrange("c (y u x v) -> c y u x v", y=32, u=2, x=32, v=2)
        t1 = wk.tile([CA, NP], F32, tag="t1", bufs=1)
        nc.vector.tensor_tensor(t1[:], fav[:, :, 0, :, 0], fav[:, :, 0, :, 1], OP.add)
        t2 = wk.tile([CA, NP], F32, tag="t2", bufs=1)
        nc.vector.tensor_tensor(t2[:], fav[:, :, 1, :, 0], fav[:, :, 1, :, 1], OP.add)
        fT32 = wk.tile([CA, NP], F32, tag="f32", bufs=1)
        nc.vector.tensor_tensor(fT32[:], t1[:], t2[:], OP.add)
        fT_bf = per.tile([CA, NP], BF16, tag="fbf", bufs=1)
        nc.vector.tensor_copy(fT_bf[:], fT32[:])
        sq = wk.tile([CA, NP], F32, tag="t1", bufs=1)
        nc.vector.tensor_tensor(sq[:], fT_bf[:], fT_bf[:], OP.mult)

        # nsq row = ones^T @ sq ; rnorm = rsqrt(nsq)
        nsq_p = pp0.tile([1, NP], F32, tag="mp")
        for ch in range(2):
            nc.tensor.matmul(nsq_p[:, ch * 512:(ch + 1) * 512],
                             ones_col_f[:], sq[:, ch * 512:(ch + 1) * 512],
                             start=True, stop=True)
        nc.scalar.sqrt(srt[:], nsq_p[:])
        nc.vector.reciprocal(rnorm_row[:], srt[:])

        # b row, its partition-broadcast, and fT2 = fT_bf * b[i]
        nc.vector.tensor_tensor(b_row[:], rnorm_row[:], pfb_row[:], OP.mult)
        bb_p = pp0.tile([128, NP], F32, tag="mp")
        for ch in range(2):
            nc.tensor.matmul(bb_p[:, ch * 512:(ch + 1) * 512],
                             ones_row[:], b_row[:, ch * 512:(ch + 1) * 512],
                             start=True, stop=True)
        fT2 = per.tile([CA, NP], BF16, tag="fT2", bufs=1)
        nc.vector.tensor_tensor(fT2[:], fT_bf[:], bb_p[:], OP.mult)

        # column forms via K=1 matmuls: pfb_col, rnorm_col -> a_col, ompfb
        pc_p = pp0.tile([128, 16], F32, tag="mp")
        for jb in range(8):
            nc.tensor.matmul(pc_p[:, jb:jb + 1],
                             pfb_row[:, jb * 128:(jb + 1) * 128],
                             ones_row[:, 0:1], start=True, stop=True)
            nc.tensor.matmul(pc_p[:, 8 + jb:9 + jb],
                             rnorm_row[:, jb * 128:(jb + 1) * 128],
                             ones_row[:, 0:1], start=True, stop=True)
        ompfb_col = per.tile([128, 8], F32, tag="omp", bufs=1)
        nc.vector.tensor_scalar(ompfb_col[:], pc_p[:, 0:8], -1.0, 1.0,
                                OP.mult, OP.add)
        a_col = per.tile([128, 8], F32, tag="acol", bufs=1)
        nc.vector.tensor_tensor(a_col[:], ompfb_col[:], pc_p[:, 8:16], OP.mult)

        # sim + exp + D + sT per j-block
        sT = []
        D_p = dp.tile([1, NP], F32)
        for jb in range(8):
            simp = sp.tile([128, NP], F32, tag="sim")
            for ch in range(2):
                nc.tensor.matmul(simp[:, ch * 512:(ch + 1) * 512],
                                 fT_bf[:, jb * 128:(jb + 1) * 128],
                                 fT2[:, ch * 512:(ch + 1) * 512],
                                 start=True, stop=True)
            Ej = wk.tile([128, NP], BF16, tag="Ej", bufs=2)
            nc.scalar.activation(Ej[:], simp[:], ACT.Exp,
                                 scale=a_col[:, jb:jb + 1])
            for ch in range(2):
                nc.tensor.matmul(D_p[:, ch * 512:(ch + 1) * 512],
                                 ones_col_b[:],
                                 Ej[:, ch * 512:(ch + 1) * 512],
                                 start=(jb == 0), stop=(jb == 7))
            st = stp.tile([128, NP], BF16, tag="sT")
            nc.vector.tensor_scalar(st[:], Ej[:],
                                    ompfb_col[:, jb:jb + 1], None, OP.mult)
            sT.append(st)

        # g_col = pfb / D
        nc.vector.tensor_copy(dsb[:], D_p[:])
        nc.vector.reciprocal(rdr[:], dsb[:])
        nc.vector.tensor_tensor(g_row[:], rdr[:], pfb_row[:], OP.mult)
        g_p = pp0.tile([128, 8], F32, tag="mp")
        for jb in range(8):
            nc.tensor.matmul(g_p[:, jb:jb + 1],
                             g_row[:, jb * 128:(jb + 1) * 128],
                             ones_row[:, 0:1], start=True, stop=True)
        g_col = per.tile([128, 8], F32, tag="gcol")
        nc.vector.tensor_copy(g_col[:], g_p[:])

    state[b].update({"sT": sT, "g_col": g_col})


def _emit_main(nc, b, io, state, mp, op_, out_dev):
    """Phase 2: out[i, d] = (sum_j sT fp) * g, d-chunk-major for early
    fp-tile release (enables next batch's prefetch)."""
    sT = state[b]["sT"]
    fpt = state[b]["fpt"]
    g_col = state[b]["g_col"]
    for dq in range(8):
        for ib in range(8):
            acc = mp.tile([128, 512], F32, tag="acc")
            for jb in range(8):
                ft = fpt[jb * 4 + dq // 2]
                nc.tensor.matmul(
                    acc[:],
                    sT[jb][:, ib * 128:(ib + 1) * 128],
                    ft[:, (dq % 2) * 512:(dq % 2) * 512 + 512],
                    start=(jb == 0), stop=(jb == 7))
            ot = op_.tile([128, 512], F32, tag="ot")
            nc.vector.tensor_scalar(ot[:], acc[:],
                                    g_col[:, ib:ib + 1], None, OP.mult)
            nc.sync.dma_start(
                out_dev[b, ib * 128:(ib + 1) * 128,
                        dq * 512:(dq + 1) * 512], ot[:])


def build_program():
    nc = bacc.Bacc("TRN2", target_bir_lowering=False, debug=False,
                   num_devices=N_CORES)
    fp_in = nc.dram_tensor("fp_in", [BPC, NP, D], BF16, kind="ExternalInput")
    fa_in = nc.dram_tensor("fa_in", [BPC, CA, 4096], F32, kind="ExternalInput")
    mask_in = nc.dram_tensor("mask_in", [BPC, 256, 256], F32, kind="ExternalInput")
    out_dev = nc.dram_tensor("out_dev", [BPC, NP, D], F32, kind="ExternalOutput")
    io = (fp_in, fa_in, mask_in, out_dev)

    with tile.TileContext(nc) as tc:
        with tc.tile_pool(name="fpp", bufs=37) as fpp, \
             tc.tile_pool(name="ldp", bufs=1) as ldp, \
             tc.tile_pool(name="stp", bufs=16) as stp, \
             tc.tile_pool(name="per", bufs=2) as per, \
             tc.tile_pool(name="wk", bufs=2) as wk, \
             tc.tile_pool(name="cst", bufs=1) as cst:
            ones_col_f = cst.tile([128, 1], F32, tag="c1")
            nc.vector.memset(ones_col_f[:], 1.0)
            ones_col_b = cst.tile([128, 1], BF16, tag="c2")
            nc.vector.memset(ones_col_b[:], 1.0)
            ones_row = cst.tile([1, 128], F32, tag="c3")
            nc.vector.memset(ones_row[:], 1.0)
            consts = (ones_col_f, ones_col_b, ones_row)
            pools = (fpp, ldp, stp, per, wk, cst)

            state = {}
            _emit_loads(nc, 0, io, pools, state)
            _emit_softmax(nc, tc, 0, pools, state, consts)
            _emit_loads(nc, 1, io, pools, state)
            _emit_softmax(nc, tc, 1, pools, state, consts)
            with tc.tile_pool(name="mm", bufs=2, space="PSUM") as mp, \
                 tc.tile_pool(name="ot", bufs=3) as op_:
                _emit_main(nc, 0, io, state, mp, op_, out_dev)
                _emit_main(nc, 1, io, state, mp, op_, out_dev)
    nc.compile()
    return nc


_NC_CACHE = None


def _get_nc():
    global _NC_CACHE
    if _NC_CACHE is None:
        _NC_CACHE = build_program()
    return _NC_CACHE


def kernel(feature, feature_attn, mask):
    feature = np.asarray(feature)
    feature_attn = np.asarray(feature_attn)
    mask = np.asarray(mask)
    B, c, h, w = feature.shape

    # host-side patch gather (pure permutation) + bf16 cast
    fp = (feature.reshape(B, c, P, 8, P, 8)
          .transpose(0, 2, 4, 3, 5, 1)
          .reshape(B, NP, D)
          .astype(ml_dtypes.bfloat16))
    fa = np.ascontiguousarray(feature_attn.reshape(B, CA, 4096))
    msk = np.ascontiguousarray(mask.reshape(B, 256, 256))

    nc = _get_nc()
    in_maps = [
        {
            "fp_in": np.ascontiguousarray(fp[i * BPC:(i + 1) * BPC]),
            "fa_in": fa[i * BPC:(i + 1) * BPC],
            "mask_in": msk[i * BPC:(i + 1) * BPC],
        }
        for i in range(N_CORES)
    ]
    res = run_bass_kernel_spmd(nc, in_maps, core_ids=list(range(N_CORES)))
    out = np.concatenate([r["out_dev"] for r in res.results], axis=0)

    # host-side inverse scatter back to [B, c, h, w]
    return (out.reshape(B, P, P, 8, 8, c)
            .transpose(0, 5, 1, 3, 2, 4)
            .reshape(B, c, h, w)
            .astype(np.float32))


# revision 8
# speedup vs baseline: 1.0220x; 1.0220x over previous
"""Trainium2 Bass kernel for nn_CAM_85770496901546 (sparse_attention).

Data-parallel over batch: 16 batch elements -> 8 cores x 2.

Per batch element (P=32 patch grid, 8x8 patches, c=64 channels):
  pfb   = maxpool8x8(mask)                      [1024]
  f     = avgpool2x2(feature_attn) flattened    [128 c, 1024 patches] (x0.25
          scale omitted: cancels in cosine normalization)
  cmat  = cos(i,j) * pfb[i] * (1-pfb[j])
  s     = softmax_j(cmat) * p_matrix
  out   = s @ fp,  fp = patch-gathered feature  [1024 j, 4096 d]

Everything on device is computed in the transposed [j, i] layout so softmax
denominators / per-i factors fold into matmuls and PSUM evacuation (no
on-chip transposes at all):
  fT2[c,i]  = fT_bf[c,i] * b[i], b = rnorm*pfb  (b broadcast via K=1 matmul;
              folding it into f makes sim2 = f^T fT2 = sim * b[i] directly)
  E[j,i]    = exp(sim2 * a[j]),  a = rnorm*(1-pfb) as per-partition ACT scale
              (ACT reads the sim2 PSUM tile directly, writes bf16)
  D[i]      = sum_j E  (ones-column matmul, fp32 PSUM accumulation)
  sT_eff    = E * (1-pfb[j])            (per-partition tensor_scalar, bf16)
  out[i,d]  = (sum_j sT_eff[j,i] fp[j,d]) * (pfb[i]/D[i])  <- folded into the
              PSUM->SBUF evacuation tensor_scalar

Phase ordering keeps TensorE dense for HAM warmth: prep+softmax for BOTH
batch elements runs before/overlapping the two back-to-back main-matmul
blocks (batch 1's softmax overlaps batch 0's main matmul; PSUM is split
2 banks for the matmul accumulators + 6 banks for the softmax pipeline).

The patch gather of `feature` -> fp[j, d] and the inverse scatter of the
output are pure data-movement permutations of the sharding layer; they are
done on host in numpy (fp is also pre-cast to bf16 there, halving its HBM
footprint). Exp needs no max-subtraction: |cmat| <= 1 by construction.
"""

import numpy as np
import ml_dtypes

import concourse.bacc as bacc
import concourse.tile as tile
import concourse.mybir as mybir
from concourse.bass_utils import run_bass_kernel_spmd

F32 = mybir.dt.float32
BF16 = mybir.dt.bfloat16
AX = mybir.AxisListType
OP = mybir.AluOpType
ACT = mybir.ActivationFunctionType

N_CORES = 8
BPC = 2          # batch elements per core
P = 32           # patch grid
NP = P * P       # 1024 patches
C = 64           # feature channels
D = 4096         # ph*pw*c
CA = 128         # attn channels


def _emit_loads(nc, b, io, pools, state):
    fp_in, fa_in, mask_in, out_dev = io
    fpp, ldp, stp, per, wk, cst = pools
    mask_t = ldp.tile([32, 2048], F32, tag="mask", bufs=1)
    nc.sync.dma_start(mask_t[:], mask_in[b].rearrange("(a q) w -> a (q w)", q=8))
    fa_t = ldp.tile([CA, 4096], F32, tag="fa", bufs=1)
    nc.sync.dma_start(fa_t[:], fa_in[b])
    fpt = []
    for jb in range(8):
        for q in range(4):
            t = fpp.tile([128, 1024], BF16, tag="fp")
            nc.sync.dma_start(
                t[:], fp_in[b, jb * 128:(jb + 1) * 128,
                             q * 1024:(q + 1) * 1024])
            fpt.append(t)  # index jb*4 + q
    state[b] = {"mask_t": mask_t, "fa_t": fa_t, "fpt": fpt}


def _emit_softmax(nc, tc, b, pools, state, consts):
    """Phase 0+1: pfb, f, sim, exp, D, sT, g."""
    fpp, ldp, stp, per, wk, cst = pools
    ones_col_f, ones_col_b, ones_row = consts
    st_ = state[b]
    mask_t, fa_t = st_["mask_t"], st_["fa_t"]

    with tc.tile_pool(name=f"pp0_{b}", bufs=1, space="PSUM") as pp0, \
         tc.tile_pool(name=f"p1s_{b}", bufs=(2 if b == 0 else 1),
                      space="PSUM") as sp, \
         tc.tile_pool(name=f"p1d_{b}", bufs=1, space="PSUM") as dp:
        # row vectors (separate tiles: matmul operands need base partition 0)
        pfb_row = per.tile([1, NP], F32, tag="pfbr", bufs=1)
        rnorm_row = per.tile([1, NP], F32, tag="rnr", bufs=1)
        b_row = per.tile([1, NP], F32, tag="brow", bufs=1)
        g_row = per.tile([1, NP], F32, tag="grow", bufs=1)
        srt = wk.tile([1, NP], F32, tag="srt", bufs=1)
        dsb = wk.tile([1, NP], F32, tag="dsb", bufs=1)
        rdr = wk.tile([1, NP], F32, tag="rdr", bufs=1)

        # mask maxpool -> pfb row
        m1 = wk.tile([32, 256], F32, tag="m1", bufs=1)
        nc.vector.tensor_reduce(
            m1[:], mask_t.rearrange("p (ph w) -> p w ph", ph=8), AX.X, OP.max)
        pfb2d = wk.tile([32, 32], F32, tag="m2", bufs=1)
        nc.vector.tensor_reduce(
            pfb2d[:], m1.rearrange("p (pw q) -> p pw q", q=8), AX.X, OP.max)
        nc.sync.dma_start(pfb_row[:], pfb2d[:])

        # feature_attn avgpool (no 0.25 scale) + bf16 cast + squares
        fav = fa_t.rearrange("c (y u x v) -> c y u x v", y=32, u=2, x=32, v=2)
        t1 = wk.tile([CA, NP], F32, tag="t1", bufs=1)
        nc.vector.tensor_tensor(t1[:], fav[:, :, 0, :, 0], fav[:, :, 0, :, 1], OP.add)
        t2 = wk.tile([CA, NP], F32, tag="t2", bufs=1)
        nc.vector.tensor_tensor(t2[:], fav[:, :, 1, :, 0], fav[:, :, 1, :, 1], OP.add)
        fT32 = wk.tile([CA, NP], F32, tag="f32", bufs=1)
        nc.vector.tensor_tensor(fT32[:], t1[:], t2[:], OP.add)
        fT_bf = per.tile([CA, NP], BF16, tag="fbf", bufs=1)
        nc.vector.tensor_copy(fT_bf[:], fT32[:])
        sq = wk.tile([CA, NP], F32, tag="t1", bufs=1)
        nc.vector.tensor_tensor(sq[:], fT_bf[:], fT_bf[:], OP.mult)

        # nsq row = ones^T @ sq ; rnorm = rsqrt(nsq)
        nsq_p = pp0.tile([1, NP], F32, tag="mp")
        for ch in range(2):
            nc.tensor.matmul(nsq_p[:, ch * 512:(ch + 1) * 512],
                             ones_col_f[:], sq[:, ch * 512:(ch + 1) * 512],
                             start=True, stop=True)
        nc.scalar.sqrt(srt[:], nsq_p[:])
        nc.vector.reciprocal(rnorm_row[:], srt[:])

        # b row, its partition-broadcast, and fT2 = fT_bf * b[i]
        nc.vector.tensor_tensor(b_row[:], rnorm_row[:], pfb_row[:], OP.mult)
        bb_p = pp0.tile([128, NP], F32, tag="mp")
        for ch in range(2):
            nc.tensor.matmul(bb_p[:, ch * 512:(ch + 1) * 512],
                             ones_row[:], b_row[:, ch * 512:(ch + 1) * 512],
                             start=True, stop=True)
        fT2 = per.tile([CA, NP], BF16, tag="fT2", bufs=1)
        nc.vector.tensor_tensor(fT2[:], fT_bf[:], bb_p[:], OP.mult)

        # column forms via K=1 matmuls: pfb_col, rnorm_col -> a_col, ompfb
        pc_p = pp0.tile([128, 16], F32, tag="mp")
        for jb in range(8):
            nc.tensor.matmul(pc_p[:, jb:jb + 1],
                             pfb_row[:, jb * 128:(jb + 1) * 128],
                             ones_row[:, 0:1], start=True, stop=True)
            nc.tensor.matmul(pc_p[:, 8 + jb:9 + jb],
                             rnorm_row[:, jb * 128:(jb + 1) * 128],
                             ones_row[:, 0:1], start=True, stop=True)
        ompfb_col = per.tile([128, 8], F32, tag="omp", bufs=1)
        nc.vector.tensor_scalar(ompfb_col[:], pc_p[:, 0:8], -1.0, 1.0,
                                OP.mult, OP.add)
        a_col = per.tile([128, 8], F32, tag="acol", bufs=1)
        nc.vector.tensor_tensor(a_col[:], ompfb_col[:], pc_p[:, 8:16], OP.mult)

        # sim + exp + D + sT per j-block
        sT = []
        D_p = dp.tile([1, NP], F32)
        for jb in range(8):
            simp = sp.tile([128, NP], F32, tag="sim")
            for ch in range(2):
                nc.tensor.matmul(simp[:, ch * 512:(ch + 1) * 512],
                                 fT_bf[:, jb * 128:(jb + 1) * 128],
                                 fT2[:, ch * 512:(ch + 1) * 512],
                                 start=True, stop=True)
            Ej = wk.tile([128, NP], BF16, tag="Ej", bufs=2)
            nc.scalar.activation(Ej[:], simp[:], ACT.Exp,
                                 scale=a_col[:, jb:jb + 1])
            for ch in range(2):
                nc.tensor.matmul(D_p[:, ch * 512:(ch + 1) * 512],
                                 ones_col_b[:],
                                 Ej[:, ch * 512:(ch + 1) * 512],
                                 start=(jb == 0), stop=(jb == 7))
            st = stp.tile([128, NP], BF16, tag="sT")
            nc.vector.tensor_scalar(st[:], Ej[:],
                                    ompfb_col[:, jb:jb + 1], None, OP.mult)
            sT.append(st)

        # g_col = pfb / D
        nc.vector.tensor_copy(dsb[:], D_p[:])
        nc.vector.reciprocal(rdr[:], dsb[:])
        nc.vector.tensor_tensor(g_row[:], rdr[:], pfb_row[:], OP.mult)
        g_p = pp0.tile([128, 8], F32, tag="mp")
        for jb in range(8):
            nc.tensor.matmul(g_p[:, jb:jb + 1],
                             g_row[:, jb * 128:(jb + 1) * 128],
                             ones_row[:, 0:1], start=True, stop=True)
        g_col = per.tile([128, 8], F32, tag="gcol")
        nc.vector.tensor_copy(g_col[:], g_p[:])

    state[b].update({"sT": sT, "g_col": g_col})


def _emit_main(nc, b, io, state, mp, op_, out_dev):
    """Phase 2: out[i, d] = (sum_j sT fp) * g, d-chunk-major for early
    fp-tile release (enables next batch's prefetch)."""
    sT = state[b]["sT"]
    fpt = state[b]["fpt"]
    g_col = state[b]["g_col"]
    for dq in range(8):
        for ib in range(8):
            acc = mp.tile([128, 512], F32, tag="acc")
            for jb in range(8):
                ft = fpt[jb * 4 + dq // 2]
                nc.tensor.matmul(
                    acc[:],
                    sT[jb][:, ib * 128:(ib + 1) * 128],
                    ft[:, (dq % 2) * 512:(dq % 2) * 512 + 512],
                    start=(jb == 0), stop=(jb == 7))
            ot = op_.tile([128, 512], F32, tag="ot")
            nc.vector.tensor_scalar(ot[:], acc[:],
                                    g_col[:, ib:ib + 1], None, OP.mult)
            nc.scalar.dma_start(
                out_dev[b, ib * 128:(ib + 1) * 128,
                        dq * 512:(dq + 1) * 512], ot[:])


def build_program():
    nc = bacc.Bacc("TRN2", target_bir_lowering=False, debug=False,
                   num_devices=N_CORES)
    fp_in = nc.dram_tensor("fp_in", [BPC, NP, D], BF16, kind="ExternalInput")
    fa_in = nc.dram_tensor("fa_in", [BPC, CA, 4096], F32, kind="ExternalInput")
    mask_in = nc.dram_tensor("mask_in", [BPC, 256, 256], F32, kind="ExternalInput")
    out_dev = nc.dram_tensor("out_dev", [BPC, NP, D], F32, kind="ExternalOutput")
    io = (fp_in, fa_in, mask_in, out_dev)

    with tile.TileContext(nc) as tc:
        with tc.tile_pool(name="fpp", bufs=37) as fpp, \
             tc.tile_pool(name="ldp", bufs=1) as ldp, \
             tc.tile_pool(name="stp", bufs=16) as stp, \
             tc.tile_pool(name="per", bufs=2) as per, \
             tc.tile_pool(name="wk", bufs=2) as wk, \
             tc.tile_pool(name="cst", bufs=1) as cst:
            ones_col_f = cst.tile([128, 1], F32, tag="c1")
            nc.vector.memset(ones_col_f[:], 1.0)
            ones_col_b = cst.tile([128, 1], BF16, tag="c2")
            nc.vector.memset(ones_col_b[:], 1.0)
            ones_row = cst.tile([1, 128], F32, tag="c3")
            nc.vector.memset(ones_row[:], 1.0)
            consts = (ones_col_f, ones_col_b, ones_row)
            pools = (fpp, ldp, stp, per, wk, cst)

            state = {}
            _emit_loads(nc, 0, io, pools, state)
            _emit_softmax(nc, tc, 0, pools, state, consts)
            _emit_loads(nc, 1, io, pools, state)
            _emit_softmax(nc, tc, 1, pools, state, consts)
            with tc.tile_pool(name="mm", bufs=2, space="PSUM") as mp, \
                 tc.tile_pool(name="ot", bufs=3) as op_:
                _emit_main(nc, 0, io, state, mp, op_, out_dev)
                _emit_main(nc, 1, io, state, mp, op_, out_dev)
    nc.compile()
    return nc


_NC_CACHE = None


def _get_nc():
    global _NC_CACHE
    if _NC_CACHE is None:
        _NC_CACHE = build_program()
    return _NC_CACHE


def kernel(feature, feature_attn, mask):
    feature = np.asarray(feature)
    feature_attn = np.asarray(feature_attn)
    mask = np.asarray(mask)
    B, c, h, w = feature.shape

    # host-side patch gather (pure permutation) + bf16 cast
    fp = (feature.reshape(B, c, P, 8, P, 8)
          .transpose(0, 2, 4, 3, 5, 1)
          .reshape(B, NP, D)
          .astype(ml_dtypes.bfloat16))
    fa = np.ascontiguousarray(feature_attn.reshape(B, CA, 4096))
    msk = np.ascontiguousarray(mask.reshape(B, 256, 256))

    nc = _get_nc()
    in_maps = [
        {
            "fp_in": np.ascontiguousarray(fp[i * BPC:(i + 1) * BPC]),
            "fa_in": fa[i * BPC:(i + 1) * BPC],
            "mask_in": msk[i * BPC:(i + 1) * BPC],
        }
        for i in range(N_CORES)
    ]
    res = run_bass_kernel_spmd(nc, in_maps, core_ids=list(range(N_CORES)))
    out = np.concatenate([r["out_dev"] for r in res.results], axis=0)

    # host-side inverse scatter back to [B, c, h, w]
    return (out.reshape(B, P, P, 8, 8, c)
            .transpose(0, 5, 1, 3, 2, 4)
            .reshape(B, c, h, w)
            .astype(np.float32))


# revision 9
# speedup vs baseline: 1.0985x; 1.0748x over previous
"""Trainium2 Bass kernel for nn_CAM_85770496901546 (sparse_attention).

Data-parallel over batch: 16 batch elements -> 8 cores x 2.

Per batch element (P=32 patch grid, 8x8 patches, c=64 channels):
  pfb   = maxpool8x8(mask)                      [1024]
  f     = avgpool2x2(feature_attn) flattened    [128 c, 1024 patches] (x0.25
          scale omitted: cancels in cosine normalization)
  cmat  = cos(i,j) * pfb[i] * (1-pfb[j])
  s     = softmax_j(cmat) * p_matrix
  out   = s @ fp,  fp = patch-gathered feature  [1024 j, 4096 d]

Everything on device is computed in the transposed [j, i] layout so softmax
denominators / per-i factors fold into matmuls and PSUM evacuation (no
on-chip transposes at all):
  fT2[c,i]  = fT_bf[c,i] * b[i], b = rnorm*pfb  (b broadcast via K=1 matmul;
              folding it into f makes sim2 = f^T fT2 = sim * b[i] directly)
  E[j,i]    = exp(sim2 * a[j]),  a = rnorm*(1-pfb) as per-partition ACT scale
              (ACT reads the sim2 PSUM tile directly, writes bf16)
  D[i]      = sum_j E  (ones-column matmul, fp32 PSUM accumulation)
  sT_eff    = E * (1-pfb[j])            (per-partition tensor_scalar, bf16)
  out[i,d]  = (sum_j sT_eff[j,i] fp[j,d]) * (pfb[i]/D[i])  <- folded into the
              PSUM->SBUF evacuation tensor_scalar

Phase ordering keeps TensorE dense for HAM warmth: prep+softmax for BOTH
batch elements runs before/overlapping the two back-to-back main-matmul
blocks (batch 1's softmax overlaps batch 0's main matmul; PSUM is split
2 banks for the matmul accumulators + 6 banks for the softmax pipeline).

The patch gather of `feature` -> fp[j, d] and the inverse scatter of the
output are pure data-movement permutations of the sharding layer; they are
done on host in numpy (fp is also pre-cast to bf16 there, halving its HBM
footprint). Exp needs no max-subtraction: |cmat| <= 1 by construction.
"""

import numpy as np
import ml_dtypes

import concourse.bacc as bacc
import concourse.tile as tile
import concourse.mybir as mybir
from concourse.bass_utils import run_bass_kernel_spmd

F32 = mybir.dt.float32
BF16 = mybir.dt.bfloat16
AX = mybir.AxisListType
OP = mybir.AluOpType
ACT = mybir.ActivationFunctionType

N_CORES = 8
BPC = 2          # batch elements per core
P = 32           # patch grid
NP = P * P       # 1024 patches
C = 64           # feature channels
D = 4096         # ph*pw*c
CA = 128         # attn channels


def _emit_loads(nc, b, io, pools, state):
    fp_in, fa_in, mask_in, out_dev = io
    fpp, ldp, stp, per, wk, cst = pools
    mask_t = ldp.tile([32, 2048], F32, tag="mask", bufs=1)
    nc.sync.dma_start(mask_t[:], mask_in[b].rearrange("(a q) w -> a (q w)", q=8))
    fa_t = ldp.tile([CA, 4096], F32, tag="fa", bufs=1)
    nc.sync.dma_start(fa_t[:], fa_in[b])
    fpt = []
    for jb in range(8):
        for q in range(4):
            t = fpp.tile([128, 1024], BF16, tag="fp")
            nc.sync.dma_start(
                t[:], fp_in[b, jb * 128:(jb + 1) * 128,
                             q * 1024:(q + 1) * 1024])
            fpt.append(t)  # index jb*4 + q
    state[b] = {"mask_t": mask_t, "fa_t": fa_t, "fpt": fpt}


def _emit_softmax(nc, tc, b, pools, state, consts):
    """Phase 0+1: pfb, f, sim, exp, D, sT, g."""
    fpp, ldp, stp, per, wk, cst = pools
    ones_col_f, ones_col_b, ones_row = consts
    st_ = state[b]
    mask_t, fa_t = st_["mask_t"], st_["fa_t"]

    with tc.tile_pool(name=f"pp0_{b}", bufs=1, space="PSUM") as pp0, \
         tc.tile_pool(name=f"p1s_{b}", bufs=(2 if b == 0 else 1),
                      space="PSUM") as sp, \
         tc.tile_pool(name=f"p1d_{b}", bufs=1, space="PSUM") as dp:
        # row vectors (separate tiles: matmul operands need base partition 0)
        pfb_row = per.tile([1, NP], F32, tag="pfbr", bufs=1)
        rnorm_row = per.tile([1, NP], F32, tag="rnr", bufs=1)
        b_row = per.tile([1, NP], F32, tag="brow", bufs=1)
        g_row = per.tile([1, NP], F32, tag="grow", bufs=1)
        srt = wk.tile([1, NP], F32, tag="srt", bufs=1)
        dsb = wk.tile([1, NP], F32, tag="dsb", bufs=1)
        rdr = wk.tile([1, NP], F32, tag="rdr", bufs=1)

        # mask maxpool -> pfb row
        m1 = wk.tile([32, 256], F32, tag="m1", bufs=1)
        nc.vector.tensor_reduce(
            m1[:], mask_t.rearrange("p (ph w) -> p w ph", ph=8), AX.X, OP.max)
        pfb2d = wk.tile([32, 32], F32, tag="m2", bufs=1)
        nc.vector.tensor_reduce(
            pfb2d[:], m1.rearrange("p (pw q) -> p pw q", q=8), AX.X, OP.max)
        nc.sync.dma_start(pfb_row[:], pfb2d[:])

        # feature_attn avgpool (no 0.25 scale) + bf16 cast + squares
        fav = fa_t.rearrange("c (y u x v) -> c y u x v", y=32, u=2, x=32, v=2)
        t1 = wk.tile([CA, NP], F32, tag="t1", bufs=1)
        nc.vector.tensor_tensor(t1[:], fav[:, :, 0, :, 0], fav[:, :, 0, :, 1], OP.add)
        t2 = wk.tile([CA, NP], F32, tag="t2", bufs=1)
        nc.vector.tensor_tensor(t2[:], fav[:, :, 1, :, 0], fav[:, :, 1, :, 1], OP.add)
        fT32 = wk.tile([CA, NP], F32, tag="f32", bufs=1)
        nc.vector.tensor_tensor(fT32[:], t1[:], t2[:], OP.add)
        fT_bf = per.tile([CA, NP], BF16, tag="fbf", bufs=1)
        nc.vector.tensor_copy(fT_bf[:], fT32[:])
        sq = wk.tile([CA, NP], F32, tag="t1", bufs=1)
        nc.vector.tensor_tensor(sq[:], fT_bf[:], fT_bf[:], OP.mult)

        # nsq row = ones^T @ sq ; rnorm = rsqrt(nsq)
        nsq_p = pp0.tile([1, NP], F32, tag="mp")
        for ch in range(2):
            nc.tensor.matmul(nsq_p[:, ch * 512:(ch + 1) * 512],
                             ones_col_f[:], sq[:, ch * 512:(ch + 1) * 512],
                             start=True, stop=True)
        nc.scalar.sqrt(srt[:], nsq_p[:])
        nc.vector.reciprocal_approx_fast(rnorm_row[:], srt[:])

        # b row, its partition-broadcast, and fT2 = fT_bf * b[i]
        nc.vector.tensor_tensor(b_row[:], rnorm_row[:], pfb_row[:], OP.mult)
        bb_p = pp0.tile([128, NP], F32, tag="mp")
        for ch in range(2):
            nc.tensor.matmul(bb_p[:, ch * 512:(ch + 1) * 512],
                             ones_row[:], b_row[:, ch * 512:(ch + 1) * 512],
                             start=True, stop=True)
        fT2 = per.tile([CA, NP], BF16, tag="fT2", bufs=1)
        nc.vector.tensor_tensor(fT2[:], fT_bf[:], bb_p[:], OP.mult)

        # column forms via K=1 matmuls: pfb_col, rnorm_col -> a_col, ompfb
        pc_p = pp0.tile([128, 16], F32, tag="mp")
        for jb in range(8):
            nc.tensor.matmul(pc_p[:, jb:jb + 1],
                             pfb_row[:, jb * 128:(jb + 1) * 128],
                             ones_row[:, 0:1], start=True, stop=True)
            nc.tensor.matmul(pc_p[:, 8 + jb:9 + jb],
                             rnorm_row[:, jb * 128:(jb + 1) * 128],
                             ones_row[:, 0:1], start=True, stop=True)
        ompfb_col = per.tile([128, 8], F32, tag="omp", bufs=1)
        nc.vector.tensor_scalar(ompfb_col[:], pc_p[:, 0:8], -1.0, 1.0,
                                OP.mult, OP.add)
        a_col = per.tile([128, 8], F32, tag="acol", bufs=1)
        nc.vector.tensor_tensor(a_col[:], ompfb_col[:], pc_p[:, 8:16], OP.mult)

        # sim + exp + D + sT per j-block
        sT = []
        D_p = dp.tile([1, NP], F32)
        for jb in range(8):
            simp = sp.tile([128, NP], F32, tag="sim")
            for ch in range(2):
                nc.tensor.matmul(simp[:, ch * 512:(ch + 1) * 512],
                                 fT_bf[:, jb * 128:(jb + 1) * 128],
                                 fT2[:, ch * 512:(ch + 1) * 512],
                                 start=True, stop=True)
            Ej = wk.tile([128, NP], BF16, tag="Ej", bufs=2)
            nc.scalar.activation(Ej[:], simp[:], ACT.Exp,
                                 scale=a_col[:, jb:jb + 1])
            for ch in range(2):
                nc.tensor.matmul(D_p[:, ch * 512:(ch + 1) * 512],
                                 ones_col_b[:],
                                 Ej[:, ch * 512:(ch + 1) * 512],
                                 start=(jb == 0), stop=(jb == 7))
            st = stp.tile([128, NP], BF16, tag="sT")
            nc.vector.tensor_scalar(st[:], Ej[:],
                                    ompfb_col[:, jb:jb + 1], None, OP.mult)
            sT.append(st)

        # g_col = pfb / D
        nc.vector.tensor_copy(dsb[:], D_p[:])
        nc.vector.reciprocal_approx_fast(rdr[:], dsb[:])
        nc.vector.tensor_tensor(g_row[:], rdr[:], pfb_row[:], OP.mult)
        g_p = pp0.tile([128, 8], F32, tag="mp")
        for jb in range(8):
            nc.tensor.matmul(g_p[:, jb:jb + 1],
                             g_row[:, jb * 128:(jb + 1) * 128],
                             ones_row[:, 0:1], start=True, stop=True)
        g_col = per.tile([128, 8], F32, tag="gcol")
        nc.vector.tensor_copy(g_col[:], g_p[:])

    state[b].update({"sT": sT, "g_col": g_col})


def _emit_main(nc, b, io, state, mp, op_, out_dev):
    """Phase 2: out[i, d] = (sum_j sT fp) * g, d-chunk-major for early
    fp-tile release (enables next batch's prefetch)."""
    sT = state[b]["sT"]
    fpt = state[b]["fpt"]
    g_col = state[b]["g_col"]
    for dq in range(8):
        for ib in range(8):
            acc = mp.tile([128, 512], F32, tag="acc")
            for jb in range(8):
                ft = fpt[jb * 4 + dq // 2]
                nc.tensor.matmul(
                    acc[:],
                    sT[jb][:, ib * 128:(ib + 1) * 128],
                    ft[:, (dq % 2) * 512:(dq % 2) * 512 + 512],
                    start=(jb == 0), stop=(jb == 7))
            ot = op_.tile([128, 512], F32, tag="ot")
            nc.vector.tensor_scalar(ot[:], acc[:],
                                    g_col[:, ib:ib + 1], None, OP.mult)
            nc.scalar.dma_start(
                out_dev[b, ib * 128:(ib + 1) * 128,
                        dq * 512:(dq + 1) * 512], ot[:])


def build_program():
    nc = bacc.Bacc("TRN2", target_bir_lowering=False, debug=False,
                   num_devices=N_CORES)
    fp_in = nc.dram_tensor("fp_in", [BPC, NP, D], BF16, kind="ExternalInput")
    fa_in = nc.dram_tensor("fa_in", [BPC, CA, 4096], F32, kind="ExternalInput")
    mask_in = nc.dram_tensor("mask_in", [BPC, 256, 256], F32, kind="ExternalInput")
    out_dev = nc.dram_tensor("out_dev", [BPC, NP, D], F32, kind="ExternalOutput")
    io = (fp_in, fa_in, mask_in, out_dev)

    with tile.TileContext(nc) as tc:
        with tc.tile_pool(name="fpp", bufs=37) as fpp, \
             tc.tile_pool(name="ldp", bufs=1) as ldp, \
             tc.tile_pool(name="stp", bufs=16) as stp, \
             tc.tile_pool(name="per", bufs=2) as per, \
             tc.tile_pool(name="wk", bufs=2) as wk, \
             tc.tile_pool(name="cst", bufs=1) as cst:
            ones_col_f = cst.tile([128, 1], F32, tag="c1")
            nc.vector.memset(ones_col_f[:], 1.0)
            ones_col_b = cst.tile([128, 1], BF16, tag="c2")
            nc.vector.memset(ones_col_b[:], 1.0)
            ones_row = cst.tile([1, 128], F32, tag="c3")
            nc.vector.memset(ones_row[:], 1.0)
            consts = (ones_col_f, ones_col_b, ones_row)
            pools = (fpp, ldp, stp, per, wk, cst)

            state = {}
            _emit_loads(nc, 0, io, pools, state)
            _emit_softmax(nc, tc, 0, pools, state, consts)
            _emit_loads(nc, 1, io, pools, state)
            _emit_softmax(nc, tc, 1, pools, state, consts)
            with tc.tile_pool(name="mm", bufs=2, space="PSUM") as mp, \
                 tc.tile_pool(name="ot", bufs=3) as op_:
                _emit_main(nc, 0, io, state, mp, op_, out_dev)
                _emit_main(nc, 1, io, state, mp, op_, out_dev)
    nc.compile()
    return nc


_NC_CACHE = None


def _get_nc():
    global _NC_CACHE
    if _NC_CACHE is None:
        _NC_CACHE = build_program()
    return _NC_CACHE


def kernel(feature, feature_attn, mask):
    feature = np.asarray(feature)
    feature_attn = np.asarray(feature_attn)
    mask = np.asarray(mask)
    B, c, h, w = feature.shape

    # host-side patch gather (pure permutation) + bf16 cast
    fp = (feature.reshape(B, c, P, 8, P, 8)
          .transpose(0, 2, 4, 3, 5, 1)
          .reshape(B, NP, D)
          .astype(ml_dtypes.bfloat16))
    fa = np.ascontiguousarray(feature_attn.reshape(B, CA, 4096))
    msk = np.ascontiguousarray(mask.reshape(B, 256, 256))

    nc = _get_nc()
    in_maps = [
        {
            "fp_in": np.ascontiguousarray(fp[i * BPC:(i + 1) * BPC]),
            "fa_in": fa[i * BPC:(i + 1) * BPC],
            "mask_in": msk[i * BPC:(i + 1) * BPC],
        }
        for i in range(N_CORES)
    ]
    res = run_bass_kernel_spmd(nc, in_maps, core_ids=list(range(N_CORES)))
    out = np.concatenate([r["out_dev"] for r in res.results], axis=0)

    # host-side inverse scatter back to [B, c, h, w]
    return (out.reshape(B, P, P, 8, 8, c)
            .transpose(0, 5, 1, 3, 2, 4)
            .reshape(B, c, h, w)
            .astype(np.float32))
